# revision 20
# baseline (speedup 1.0000x reference)
"""Trainium2 Bass kernel for nn_BilinAndFwdComboVecComp.

Math (B=8, S=256, C=256, V=64):
  final[b,s,z,k] = tanh( sum_ij ctx[b,s,i] ctx[b,z,j] W'[i,j,k] + A[b,z,k] + Bt[b,s,k] )
where
  W'[i,j,k] = W[i,j,k] + (i==j) * linmul_w[k,i]          (folds the `mul` branch)
  A[b,z,k]  = ctx[b] @ (lin1_w+lindiff_w).T + (lin1_b + bias + linmul_b + lindiff_b)
  Bt[b,s,k] = ctx[b] @ (lin2_w-lindiff_w).T + lin2_b     (the `diff` branch is rank-1
                                                          per pair and merges into A/Bt)

Sharding: V split across the 8 cores (8 k-values per core). Each core:
  phase 1: tmp2_b[i,(k,z)] = sum_j Wt[j,(k,i)]^T-slices @ ctxT_b[j,z]   (W-stationary)
  phase 2: out[s,(z,k)]    = ctxT_b[:,s]^T @ tmp2_b[:,(z,k)]  (+ K=9 fold matmul that
           adds A via a ones-row and Bt via delta-rows), then tanh on ACT, DMA out.
All matmuls run as float32r (full PE rate at moving-dim >= 256, fp32-class precision).
Per-core output scratch (B,S,S,8) is concatenated on the host along k.
"""

import numpy as np

B, S, C, V = 8, 256, 256, 64
NCORES = 8
KV = V // NCORES  # k-values per core


def _host_prep(ctx, W, bias, lin1_w, lin1_b, lin2_w, lin2_b,
               linmul_w, linmul_b, lindiff_w, lindiff_b):
    f = np.float32
    ctx = np.asarray(ctx, f)
    Wp = np.array(W, f)
    Wp[np.arange(C), np.arange(C), :] += np.asarray(linmul_w, f).T
    Wt = Wp.transpose(1, 0, 2)  # [j, i, k]

    A = ctx @ (np.asarray(lin1_w, f) + np.asarray(lindiff_w, f)).T \
        + (np.asarray(lin1_b, f) + np.asarray(bias, f)
           + np.asarray(linmul_b, f) + np.asarray(lindiff_b, f))
    Bt = ctx @ (np.asarray(lin2_w, f) - np.asarray(lindiff_w, f)).T + np.asarray(lin2_b, f)

    ctxT = np.ascontiguousarray(ctx.transpose(0, 2, 1))  # [B, C, S]

    # delta in (k, z) layout: row r is 1 over the z-block of plane k==r
    delta = np.zeros((KV, KV * S), f)
    for r in range(KV):
        delta[r, r * S:(r + 1) * S] = 1.0

    per_core = []
    for c in range(NCORES):
        ks = slice(c * KV, (c + 1) * KV)
        # wt layout: [j, kk*C + i]
        wt = np.ascontiguousarray(Wt[:, :, ks].transpose(0, 2, 1).reshape(C, KV * C))
        # fold contraction, K = 2*KV+2 rows: Bt hi/lo via delta rows (exact in
        # fp16 as hi + residual), A hi/lo via ones rows
        Btc = Bt[:, :, ks].transpose(2, 0, 1).reshape(KV, B * S)
        Btc_hi = Btc.astype(np.float16).astype(f)
        Ac = A[:, :, ks].transpose(0, 2, 1).reshape(B, KV * S)
        Ac_hi = Ac.astype(np.float16).astype(f)
        KF = 2 * KV + 2
        foldL = np.empty((KF, B * S), f)
        foldL[:KV] = Btc_hi
        foldL[KV:2 * KV] = Btc - Btc_hi
        foldL[2 * KV:] = 1.0
        foldR = np.empty((B, KF, KV * S), f)
        foldR[:, :KV, :] = delta[None]
        foldR[:, KV:2 * KV, :] = delta[None]
        foldR[:, 2 * KV, :] = Ac_hi
        foldR[:, 2 * KV + 1, :] = Ac - Ac_hi
        per_core.append({"ctxT": ctxT, "wt": wt, "foldL": foldL, "foldR": foldR})
    import os
    if os.environ.get("KERNEL_DTYPE", "f16") == "f16":
        per_core = [{k: v.astype(np.float16) for k, v in m.items()} for m in per_core]
    return per_core


def _build_program():
    import concourse.tile as tile
    import concourse.mybir as mybir
    from concourse import bacc
    from contextlib import ExitStack

    import os
    f32 = mybir.dt.float32
    f16 = mybir.dt.float16
    if os.environ.get("KERNEL_DTYPE", "f16") == "f32r":
        f16 = mybir.dt.float32r  # compute dtype for matmul operands
    TANH = mybir.ActivationFunctionType.Tanh

    nc = bacc.Bacc("TRN2", target_bir_lowering=False, debug=False)
    ctxT_d = nc.dram_tensor("ctxT", [B, C, S], f16, kind="ExternalInput").ap()
    wt_d = nc.dram_tensor("wt", [C, KV * C], f16, kind="ExternalInput").ap()
    KF = 2 * KV + 2
    foldL_d = nc.dram_tensor("foldL", [KF, B * S], f16, kind="ExternalInput").ap()
    foldR_d = nc.dram_tensor("foldR", [B, KF, S * KV], f16, kind="ExternalInput").ap()
    # out scratch is (k, z)-ordered; the host transposes back to (z, k)
    out_d = nc.dram_tensor("out", [B, S, KV, S], mybir.dt.float16, kind="ExternalOutput").ap()

    with tile.TileContext(nc) as tc, ExitStack() as es:
        ctx_pool = es.enter_context(tc.tile_pool(name="ctxp", bufs=8))
        wt_pool = es.enter_context(tc.tile_pool(name="wtp", bufs=2))
        fl_pool = es.enter_context(tc.tile_pool(name="flp", bufs=1))
        fr_pool = es.enter_context(tc.tile_pool(name="frp", bufs=2))
        tmp2_pool = es.enter_context(tc.tile_pool(name="tmp2p", bufs=6))
        outs_pool = es.enter_context(tc.tile_pool(name="outsp", bufs=4))


        # weights + first ctx pair first so the PE starts ASAP; the kk=0
        # slice of wt arrives first so the very first matmuls can issue
        wt_sb = []
        for j in range(2):
            t = wt_pool.tile([128, KV * C], f16, name=f"wt_{j}", bufs=1)
            nc.sync.dma_start(t[:, 0:C], wt_d[j * 128:(j + 1) * 128, 0:C])
            wt_sb.append(t)
        for j in range(2):
            nc.sync.dma_start(wt_sb[j][:, C:], wt_d[j * 128:(j + 1) * 128, C:])
        # ctx pair tiles: [128, 512] = ctxT[2p, jchunk] | ctxT[2p+1, jchunk]
        ctxp_sb = {}
        for p in range(B // 2):
            for j in range(2):
                t = ctx_pool.tile([128, 2 * S], f16, name=f"ctx_{p}_{j}", bufs=1)
                nc.sync.dma_start(t[:, 0:S], ctxT_d[2 * p, j * 128:(j + 1) * 128, :])
                nc.sync.dma_start(t[:, S:2 * S], ctxT_d[2 * p + 1, j * 128:(j + 1) * 128, :])
                ctxp_sb[p, j] = t
        foldL_sb = fl_pool.tile([KF, B * S], f16, name="foldL", bufs=1)
        nc.sync.dma_start(foldL_sb[:], foldL_d[:])

        tmp2p = {}

        def phase1(pg, ps1_pool, copy_engines=("vector",)):  # pg: pair indices
            ce = [0]
            for ch in range(2):  # i-chunk (output partition of tmp2)
                for p in pg:
                    # pair tile, layout (h=b-half, k, z)
                    tmp2p[p, ch] = tmp2_pool.tile([128, 2 * KV * S], f16, name="tmp2")
                for kk in range(KV):
                    ps = {}
                    for p in pg:
                        ps[p] = ps1_pool.tile([128, 2 * S], f32, name="ps1")
                    for j in range(2):  # contraction chunk
                        lhsT = wt_sb[j][:, kk * C + ch * 128: kk * C + ch * 128 + 128]
                        for p in pg:
                            nc.tensor.matmul(
                                ps[p][:], lhsT, ctxp_sb[p, j][:],
                                start=(j == 0), stop=(j == 1),
                            )
                    for p in pg:
                        # one copy per bank: psum (h, z) -> pair tile (h, kk, z)
                        dst = tmp2p[p, ch][:].rearrange("q (h k z) -> q h k z", h=2, k=KV)
                        src_ap = ps[p][:].rearrange("q (h z) -> q h z", h=2)
                        eng = copy_engines[ce[0] % len(copy_engines)]
                        ce[0] += 1
                        if eng == "vector":
                            nc.vector.tensor_copy(dst[:, :, kk, :], src_ap)
                        else:
                            nc.scalar.copy(dst[:, :, kk, :], src_ap)

        def phase2(bg, ps2_pool):
            for b in bg:
                frt = fr_pool.tile([KF, S * KV], f16, name="foldR")
                nc.sync.dma_start(frt[:], foldR_d[b])
                for sc in range(2):
                    hoff = (b % 2) * KV * S
                    lhsT3 = foldL_sb[:, b * S + sc * 128: b * S + sc * 128 + 128]
                    for t in range(2):  # double-bank psum tiles, 2 n-chunks each
                        pst = ps2_pool.tile([128, 1024], f32, name="ps2")
                        n0 = 2 * t
                        for st in range(2):  # contraction chunk; one LDW per 2 MMs
                            lhsT = ctxp_sb[b // 2, st][:, (b % 2) * S + sc * 128:
                                                       (b % 2) * S + sc * 128 + 128]
                            for n in (n0, n0 + 1):
                                nc.tensor.matmul(
                                    pst[:, (n % 2) * 512:(n % 2) * 512 + 512], lhsT,
                                    tmp2p[b // 2, st][:, hoff + n * 512:hoff + (n + 1) * 512],
                                    start=(st == 0), stop=False,
                                )
                        for n in (n0, n0 + 1):
                            nc.tensor.matmul(
                                pst[:, (n % 2) * 512:(n % 2) * 512 + 512], lhsT3,
                                frt[:, n * 512:(n + 1) * 512],
                                start=False, stop=True,
                            )
                        ot = outs_pool.tile([128, 1024], mybir.dt.float16, name="ot")
                        nc.scalar.activation(ot[:], pst[:], TANH)
                        for hd in range(2):
                            eng = nc.sync if hd == 0 else nc.gpsimd
                            eng.dma_start(
                                out_d[b, sc * 128:(sc + 1) * 128,
                                      4 * t + 2 * hd:4 * t + 2 * hd + 2]
                                .rearrange("s k z -> s (k z)"),
                                ot[:, hd * 512:(hd + 1) * 512],
                            )

        # 2-pair phase-1 groups amortize weight loads; early phase-2 work is
        # PE fill-in while phase-1 waits on DVE drains (and vice versa later)
        with tc.tile_pool(name="ps1", bufs=4, space="PSUM") as ps1_pool, \
                tc.tile_pool(name="ps2a", bufs=2, space="PSUM") as ps2a:
            phase1([0, 1], ps1_pool, copy_engines=("vector", "scalar"))
            phase2([0], ps2a)
            phase2([1], ps2a)
            phase1([2, 3], ps1_pool, copy_engines=("vector",))
            phase2([2], ps2a)
            phase2([3], ps2a)
        with tc.tile_pool(name="ps2b", bufs=4, space="PSUM") as ps2b:
            for b in range(4, 8):
                phase2([b], ps2b)

    nc.compile()
    return nc


def _install_profile_hook():
    """Register the NTFF profile hook that the image's boot skipped
    (antenv.axon_hooks shim is missing in this container)."""
    import sys as _sys
    import types as _types
    try:
        import antenv
        if "antenv.axon_hooks" not in _sys.modules:
            m = _types.ModuleType("antenv.axon_hooks")
            _h = [None]
            m.set_axon_ntff_profile_hook = lambda h: _h.__setitem__(0, h)
            m.get_axon_ntff_profile_hook = lambda: _h[0]
            _sys.modules["antenv.axon_hooks"] = m
            antenv.axon_hooks = m
        from antenv.axon_hooks import set_axon_ntff_profile_hook, get_axon_ntff_profile_hook
        if get_axon_ntff_profile_hook() is None:
            from trn_agent_boot.trn_boot import _ntff_profile_via_ctypes
            set_axon_ntff_profile_hook(_ntff_profile_via_ctypes("/opt/axon/libaxon_pjrt.so"))
    except Exception:
        pass


def _patch_walrus_ldw_opt():
    """Enable walrus LDWEIGHTS dedup (concourse hardcodes it off). With fp32r
    matmuls walrus emits one LDWEIGHTS per matmul; repeated identical loads
    serialize against the matmul stream (same-row-group loads can't pull
    ahead), costing ~170-300ns per matmul."""
    import os
    if os.environ.get("KERNEL_LDW_OPT", "0") != "1":
        return
    import concourse.bass_utils as bu
    if getattr(bu.run_command, "_ldw_patched", False):
        return
    orig = bu.run_command

    def patched(argv, **kw):
        argv = ["--enable-ldw-opt=true" if a == "--enable-ldw-opt=false" else a
                for a in argv]
        return orig(argv, **kw)

    patched._ldw_patched = True
    bu.run_command = patched


def run(inputs, trace=False):
    """Returns (full_output, BassKernelResults)."""
    from concourse.bass_utils import run_bass_kernel_spmd

    _patch_walrus_ldw_opt()
    if trace:
        _install_profile_hook()
    per_core = _host_prep(**inputs)
    nc = _build_program()
    import os as _os
    _tc = [int(x) for x in _os.environ.get("KERNEL_TRACE_CORES", "0").split(",")]
    res = run_bass_kernel_spmd(nc, per_core, list(range(NCORES)), trace=trace,
                               trace_cores=_tc if trace else None)
    # per-core scratch is (B, S, KV, S) with k-major planes: swap to (B,S,S,KV)
    out = np.concatenate(
        [res.results[c]["out"].astype(np.float32).transpose(0, 1, 3, 2)
         for c in range(NCORES)], axis=3)
    out = np.ascontiguousarray(out)
    return out, res


def kernel(**inputs) -> np.ndarray:
    out, _ = run(inputs, trace=False)
    return out


# revision 22
# speedup vs baseline: 1.1176x; 1.1176x over previous
"""Trainium2 Bass kernel for nn_BilinAndFwdComboVecComp.

Math (B=8, S=256, C=256, V=64):
  final[b,s,z,k] = tanh( sum_ij ctx[b,s,i] ctx[b,z,j] W'[i,j,k] + A[b,z,k] + Bt[b,s,k] )
where
  W'[i,j,k] = W[i,j,k] + (i==j) * linmul_w[k,i]          (folds the `mul` branch)
  A[b,z,k]  = ctx[b] @ (lin1_w+lindiff_w).T + (lin1_b + bias + linmul_b + lindiff_b)
  Bt[b,s,k] = ctx[b] @ (lin2_w-lindiff_w).T + lin2_b     (the `diff` branch is rank-1
                                                          per pair and merges into A/Bt)

Sharding: V split across the 8 cores (8 k-values per core). Each core:
  phase 1: tmp2_b[i,(k,z)] = sum_j Wt[j,(k,i)]^T-slices @ ctxT_b[j,z]   (W-stationary)
  phase 2: out[s,(z,k)]    = ctxT_b[:,s]^T @ tmp2_b[:,(z,k)]  (+ K=9 fold matmul that
           adds A via a ones-row and Bt via delta-rows), then tanh on ACT, DMA out.
All matmuls run as float32r (full PE rate at moving-dim >= 256, fp32-class precision).
Per-core output scratch (B,S,S,8) is concatenated on the host along k.
"""

import numpy as np

B, S, C, V = 8, 256, 256, 64
NCORES = 8
KV = V // NCORES  # k-values per core


def _host_prep(ctx, W, bias, lin1_w, lin1_b, lin2_w, lin2_b,
               linmul_w, linmul_b, lindiff_w, lindiff_b):
    f = np.float32
    ctx = np.asarray(ctx, f)
    Wp = np.array(W, f)
    Wp[np.arange(C), np.arange(C), :] += np.asarray(linmul_w, f).T
    Wt = Wp.transpose(1, 0, 2)  # [j, i, k]

    A = ctx @ (np.asarray(lin1_w, f) + np.asarray(lindiff_w, f)).T \
        + (np.asarray(lin1_b, f) + np.asarray(bias, f)
           + np.asarray(linmul_b, f) + np.asarray(lindiff_b, f))
    Bt = ctx @ (np.asarray(lin2_w, f) - np.asarray(lindiff_w, f)).T + np.asarray(lin2_b, f)

    ctxT = np.ascontiguousarray(ctx.transpose(0, 2, 1))  # [B, C, S]

    # delta in (k, z) layout: row r is 1 over the z-block of plane k==r
    delta = np.zeros((KV, KV * S), f)
    for r in range(KV):
        delta[r, r * S:(r + 1) * S] = 1.0

    per_core = []
    for c in range(NCORES):
        ks = slice(c * KV, (c + 1) * KV)
        # wt layout: [j, kk*C + i]
        wt = np.ascontiguousarray(Wt[:, :, ks].transpose(0, 2, 1).reshape(C, KV * C))
        # fold contraction, K = 2*KV+2 rows: Bt hi/lo via delta rows (exact in
        # fp16 as hi + residual), A hi/lo via ones rows
        Btc = Bt[:, :, ks].transpose(2, 0, 1).reshape(KV, B * S)
        Btc_hi = Btc.astype(np.float16).astype(f)
        Ac = A[:, :, ks].transpose(0, 2, 1).reshape(B, KV * S)
        Ac_hi = Ac.astype(np.float16).astype(f)
        KF = 2 * KV + 2
        foldL = np.empty((KF, B * S), f)
        foldL[:KV] = Btc_hi
        foldL[KV:2 * KV] = Btc - Btc_hi
        foldL[2 * KV:] = 1.0
        foldR = np.empty((B, KF, KV * S), f)
        foldR[:, :KV, :] = delta[None]
        foldR[:, KV:2 * KV, :] = delta[None]
        foldR[:, 2 * KV, :] = Ac_hi
        foldR[:, 2 * KV + 1, :] = Ac - Ac_hi
        per_core.append({"ctxT": ctxT, "wt": wt, "foldL": foldL, "foldR": foldR})
    import os
    if os.environ.get("KERNEL_DTYPE", "f16") == "f16":
        per_core = [{k: v.astype(np.float16) for k, v in m.items()} for m in per_core]
    return per_core


def _build_program():
    import concourse.tile as tile
    import concourse.mybir as mybir
    from concourse import bacc
    from contextlib import ExitStack

    import os
    f32 = mybir.dt.float32
    f16 = mybir.dt.float16
    if os.environ.get("KERNEL_DTYPE", "f16") == "f32r":
        f16 = mybir.dt.float32r  # compute dtype for matmul operands
    TANH = mybir.ActivationFunctionType.Tanh

    nc = bacc.Bacc("TRN2", target_bir_lowering=False, debug=False)
    ctxT_d = nc.dram_tensor("ctxT", [B, C, S], f16, kind="ExternalInput").ap()
    wt_d = nc.dram_tensor("wt", [C, KV * C], f16, kind="ExternalInput").ap()
    KF = 2 * KV + 2
    foldL_d = nc.dram_tensor("foldL", [KF, B * S], f16, kind="ExternalInput").ap()
    foldR_d = nc.dram_tensor("foldR", [B, KF, S * KV], f16, kind="ExternalInput").ap()
    # out scratch is (k, z)-ordered; the host transposes back to (z, k)
    out_d = nc.dram_tensor("out", [B, S, KV, S], mybir.dt.float16, kind="ExternalOutput").ap()

    with tile.TileContext(nc) as tc, ExitStack() as es:
        ctx_pool = es.enter_context(tc.tile_pool(name="ctxp", bufs=8))
        wt_pool = es.enter_context(tc.tile_pool(name="wtp", bufs=2))
        fl_pool = es.enter_context(tc.tile_pool(name="flp", bufs=1))
        fr_pool = es.enter_context(tc.tile_pool(name="frp", bufs=2))
        tmp2_pool = es.enter_context(tc.tile_pool(name="tmp2p", bufs=6))
        outs_pool = es.enter_context(tc.tile_pool(name="outsp", bufs=6))


        # weights kk=0 slices first (split across two DMA sequencers) so the
        # PE starts ASAP
        wt_sb = []
        for j in range(2):
            t = wt_pool.tile([128, KV * C], f16, name=f"wt_{j}", bufs=1)
            (nc.sync if j == 0 else nc.gpsimd).dma_start(
                t[:, 0:C], wt_d[j * 128:(j + 1) * 128, 0:C])
            wt_sb.append(t)
        for j in range(2):
            (nc.sync if j == 0 else nc.gpsimd).dma_start(
                wt_sb[j][:, C:], wt_d[j * 128:(j + 1) * 128, C:])
        # ctx pair tiles: [128, 512] = ctxT[2p, jchunk] | ctxT[2p+1, jchunk]
        # pairs 0,1 first (phase-1 g0 critical path), queues interleaved
        ctxp_sb = {}
        qi = 0
        for p in range(B // 2):
            for j in range(2):
                t = ctx_pool.tile([128, 2 * S], f16, name=f"ctx_{p}_{j}", bufs=1)
                for h in range(2):
                    eng = nc.sync if qi % 2 == 0 else nc.gpsimd
                    qi += 1
                    eng.dma_start(t[:, h * S:(h + 1) * S],
                                  ctxT_d[2 * p + h, j * 128:(j + 1) * 128, :])
                ctxp_sb[p, j] = t
        foldL_sb = fl_pool.tile([KF, B * S], f16, name="foldL", bufs=1)
        nc.sync.dma_start(foldL_sb[:], foldL_d[:])

        tmp2p = {}

        def phase1(pg, ps1_pool, copy_engines=("vector",)):  # pg: pair indices
            ce = [0]
            for ch in range(2):  # i-chunk (output partition of tmp2)
                for p in pg:
                    # pair tile, layout (h=b-half, k, z)
                    tmp2p[p, ch] = tmp2_pool.tile([128, 2 * KV * S], f16, name="tmp2")
                for kk in range(KV):
                    ps = {}
                    for p in pg:
                        ps[p] = ps1_pool.tile([128, 2 * S], f32, name="ps1")
                    for j in range(2):  # contraction chunk
                        lhsT = wt_sb[j][:, kk * C + ch * 128: kk * C + ch * 128 + 128]
                        for p in pg:
                            nc.tensor.matmul(
                                ps[p][:], lhsT, ctxp_sb[p, j][:],
                                start=(j == 0), stop=(j == 1),
                            )
                    for p in pg:
                        # one copy per bank: psum (h, z) -> pair tile (h, kk, z)
                        dst = tmp2p[p, ch][:].rearrange("q (h k z) -> q h k z", h=2, k=KV)
                        src_ap = ps[p][:].rearrange("q (h z) -> q h z", h=2)
                        eng = copy_engines[ce[0] % len(copy_engines)]
                        ce[0] += 1
                        if eng == "vector":
                            nc.vector.tensor_copy(dst[:, :, kk, :], src_ap)
                        else:
                            nc.scalar.copy(dst[:, :, kk, :], src_ap)

        def phase2(bg, ps2_pool):
            for b in bg:
                frt = fr_pool.tile([KF, S * KV], f16, name="foldR")
                nc.sync.dma_start(frt[:], foldR_d[b])
                for sc in range(2):
                    hoff = (b % 2) * KV * S
                    lhsT3 = foldL_sb[:, b * S + sc * 128: b * S + sc * 128 + 128]
                    for n in range(4):  # one single-bank psum tile per k-pair
                        pst = ps2_pool.tile([128, 512], f32, name="ps2")
                        for st in range(2):  # contraction chunk over i
                            lhsT = ctxp_sb[b // 2, st][:, (b % 2) * S + sc * 128:
                                                       (b % 2) * S + sc * 128 + 128]
                            nc.tensor.matmul(
                                pst[:], lhsT,
                                tmp2p[b // 2, st][:, hoff + n * 512:hoff + (n + 1) * 512],
                                start=(st == 0), stop=False,
                            )
                        nc.tensor.matmul(
                            pst[:], lhsT3,
                            frt[:, n * 512:(n + 1) * 512],
                            start=False, stop=True,
                        )
                        ot = outs_pool.tile([128, 512], mybir.dt.float16, name="ot")
                        nc.scalar.activation(ot[:], pst[:], TANH)
                        eng = nc.sync if n % 2 == 0 else nc.gpsimd
                        eng.dma_start(
                            out_d[b, sc * 128:(sc + 1) * 128,
                                  2 * n:2 * n + 2].rearrange("s k z -> s (k z)"),
                            ot[:],
                        )

        # 2-pair phase-1 groups amortize weight loads; early phase-2 work is
        # PE fill-in while phase-1 waits on DVE drains (and vice versa later)
        with tc.tile_pool(name="ps1", bufs=4, space="PSUM") as ps1_pool, \
                tc.tile_pool(name="ps2a", bufs=4, space="PSUM") as ps2a:
            phase1([0, 1], ps1_pool, copy_engines=("vector", "scalar"))
            phase2([0], ps2a)
            phase2([1], ps2a)
            phase1([2, 3], ps1_pool, copy_engines=("vector",))
            phase2([2], ps2a)
            phase2([3], ps2a)
        with tc.tile_pool(name="ps2b", bufs=8, space="PSUM") as ps2b:
            for b in range(4, 8):
                phase2([b], ps2b)

    nc.compile()
    return nc


def _install_profile_hook():
    """Register the NTFF profile hook that the image's boot skipped
    (antenv.axon_hooks shim is missing in this container)."""
    import sys as _sys
    import types as _types
    try:
        import antenv
        if "antenv.axon_hooks" not in _sys.modules:
            m = _types.ModuleType("antenv.axon_hooks")
            _h = [None]
            m.set_axon_ntff_profile_hook = lambda h: _h.__setitem__(0, h)
            m.get_axon_ntff_profile_hook = lambda: _h[0]
            _sys.modules["antenv.axon_hooks"] = m
            antenv.axon_hooks = m
        from antenv.axon_hooks import set_axon_ntff_profile_hook, get_axon_ntff_profile_hook
        if get_axon_ntff_profile_hook() is None:
            from trn_agent_boot.trn_boot import _ntff_profile_via_ctypes
            set_axon_ntff_profile_hook(_ntff_profile_via_ctypes("/opt/axon/libaxon_pjrt.so"))
    except Exception:
        pass


def _patch_walrus_ldw_opt():
    """Enable walrus LDWEIGHTS dedup (concourse hardcodes it off). With fp32r
    matmuls walrus emits one LDWEIGHTS per matmul; repeated identical loads
    serialize against the matmul stream (same-row-group loads can't pull
    ahead), costing ~170-300ns per matmul."""
    import os
    if os.environ.get("KERNEL_LDW_OPT", "0") != "1":
        return
    import concourse.bass_utils as bu
    if getattr(bu.run_command, "_ldw_patched", False):
        return
    orig = bu.run_command

    def patched(argv, **kw):
        argv = ["--enable-ldw-opt=true" if a == "--enable-ldw-opt=false" else a
                for a in argv]
        return orig(argv, **kw)

    patched._ldw_patched = True
    bu.run_command = patched


def run(inputs, trace=False):
    """Returns (full_output, BassKernelResults)."""
    from concourse.bass_utils import run_bass_kernel_spmd

    _patch_walrus_ldw_opt()
    if trace:
        _install_profile_hook()
    per_core = _host_prep(**inputs)
    nc = _build_program()
    import os as _os
    _tc = [int(x) for x in _os.environ.get("KERNEL_TRACE_CORES", "0").split(",")]
    res = run_bass_kernel_spmd(nc, per_core, list(range(NCORES)), trace=trace,
                               trace_cores=_tc if trace else None)
    # per-core scratch is (B, S, KV, S) with k-major planes: swap to (B,S,S,KV)
    out = np.concatenate(
        [res.results[c]["out"].astype(np.float32).transpose(0, 1, 3, 2)
         for c in range(NCORES)], axis=3)
    out = np.ascontiguousarray(out)
    return out, res


def kernel(**inputs) -> np.ndarray:
    out, _ = run(inputs, trace=False)
    return out


# revision 23
# speedup vs baseline: 1.1210x; 1.0030x over previous
"""Trainium2 Bass kernel for nn_BilinAndFwdComboVecComp.

Math (B=8, S=256, C=256, V=64):
  final[b,s,z,k] = tanh( sum_ij ctx[b,s,i] ctx[b,z,j] W'[i,j,k] + A[b,z,k] + Bt[b,s,k] )
where
  W'[i,j,k] = W[i,j,k] + (i==j) * linmul_w[k,i]          (folds the `mul` branch)
  A[b,z,k]  = ctx[b] @ (lin1_w+lindiff_w).T + (lin1_b + bias + linmul_b + lindiff_b)
  Bt[b,s,k] = ctx[b] @ (lin2_w-lindiff_w).T + lin2_b     (the `diff` branch is rank-1
                                                          per pair and merges into A/Bt)

Sharding: V split across the 8 cores (8 k-values per core). Each core:
  phase 1: tmp2_b[i,(k,z)] = sum_j Wt[j,(k,i)]^T-slices @ ctxT_b[j,z]   (W-stationary)
  phase 2: out[s,(z,k)]    = ctxT_b[:,s]^T @ tmp2_b[:,(z,k)]  (+ K=9 fold matmul that
           adds A via a ones-row and Bt via delta-rows), then tanh on ACT, DMA out.
All matmuls run as float32r (full PE rate at moving-dim >= 256, fp32-class precision).
Per-core output scratch (B,S,S,8) is concatenated on the host along k.
"""

import numpy as np

B, S, C, V = 8, 256, 256, 64
NCORES = 8
KV = V // NCORES  # k-values per core


def _host_prep(ctx, W, bias, lin1_w, lin1_b, lin2_w, lin2_b,
               linmul_w, linmul_b, lindiff_w, lindiff_b):
    f = np.float32
    ctx = np.asarray(ctx, f)
    Wp = np.array(W, f)
    Wp[np.arange(C), np.arange(C), :] += np.asarray(linmul_w, f).T
    Wt = Wp.transpose(1, 0, 2)  # [j, i, k]

    A = ctx @ (np.asarray(lin1_w, f) + np.asarray(lindiff_w, f)).T \
        + (np.asarray(lin1_b, f) + np.asarray(bias, f)
           + np.asarray(linmul_b, f) + np.asarray(lindiff_b, f))
    Bt = ctx @ (np.asarray(lin2_w, f) - np.asarray(lindiff_w, f)).T + np.asarray(lin2_b, f)

    ctxT = np.ascontiguousarray(ctx.transpose(0, 2, 1))  # [B, C, S]

    # delta in (k, z) layout: row r is 1 over the z-block of plane k==r
    delta = np.zeros((KV, KV * S), f)
    for r in range(KV):
        delta[r, r * S:(r + 1) * S] = 1.0

    per_core = []
    for c in range(NCORES):
        ks = slice(c * KV, (c + 1) * KV)
        # wt layout: [j, kk*C + i]
        wt = np.ascontiguousarray(Wt[:, :, ks].transpose(0, 2, 1).reshape(C, KV * C))
        # fold contraction, K = 2*KV+2 rows: Bt hi/lo via delta rows (exact in
        # fp16 as hi + residual), A hi/lo via ones rows
        Btc = Bt[:, :, ks].transpose(2, 0, 1).reshape(KV, B * S)
        Btc_hi = Btc.astype(np.float16).astype(f)
        Ac = A[:, :, ks].transpose(0, 2, 1).reshape(B, KV * S)
        Ac_hi = Ac.astype(np.float16).astype(f)
        KF = 2 * KV + 2
        foldL = np.empty((KF, B * S), f)
        foldL[:KV] = Btc_hi
        foldL[KV:2 * KV] = Btc - Btc_hi
        foldL[2 * KV:] = 1.0
        foldR = np.empty((B, KF, KV * S), f)
        foldR[:, :KV, :] = delta[None]
        foldR[:, KV:2 * KV, :] = delta[None]
        foldR[:, 2 * KV, :] = Ac_hi
        foldR[:, 2 * KV + 1, :] = Ac - Ac_hi
        per_core.append({"ctxT": ctxT, "wt": wt, "foldL": foldL, "foldR": foldR})
    import os
    if os.environ.get("KERNEL_DTYPE", "f16") == "f16":
        per_core = [{k: v.astype(np.float16) for k, v in m.items()} for m in per_core]
    return per_core


def _build_program():
    import concourse.tile as tile
    import concourse.mybir as mybir
    from concourse import bacc
    from contextlib import ExitStack

    import os
    f32 = mybir.dt.float32
    f16 = mybir.dt.float16
    if os.environ.get("KERNEL_DTYPE", "f16") == "f32r":
        f16 = mybir.dt.float32r  # compute dtype for matmul operands
    TANH = mybir.ActivationFunctionType.Tanh

    nc = bacc.Bacc("TRN2", target_bir_lowering=False, debug=False)
    ctxT_d = nc.dram_tensor("ctxT", [B, C, S], f16, kind="ExternalInput").ap()
    wt_d = nc.dram_tensor("wt", [C, KV * C], f16, kind="ExternalInput").ap()
    KF = 2 * KV + 2
    foldL_d = nc.dram_tensor("foldL", [KF, B * S], f16, kind="ExternalInput").ap()
    foldR_d = nc.dram_tensor("foldR", [B, KF, S * KV], f16, kind="ExternalInput").ap()
    # out scratch is (k, z)-ordered; the host transposes back to (z, k)
    out_d = nc.dram_tensor("out", [B, S, KV, S], mybir.dt.float16, kind="ExternalOutput").ap()

    with tile.TileContext(nc) as tc, ExitStack() as es:
        ctx_pool = es.enter_context(tc.tile_pool(name="ctxp", bufs=8))
        wt_pool = es.enter_context(tc.tile_pool(name="wtp", bufs=2))
        fl_pool = es.enter_context(tc.tile_pool(name="flp", bufs=1))
        fr_pool = es.enter_context(tc.tile_pool(name="frp", bufs=2))
        tmp2_pool = es.enter_context(tc.tile_pool(name="tmp2p", bufs=6))
        outs_pool = es.enter_context(tc.tile_pool(name="outsp", bufs=6))


        # weights kk=0 slices first (split across two DMA sequencers) so the
        # PE starts ASAP
        wt_sb = []
        for j in range(2):
            t = wt_pool.tile([128, KV * C], f16, name=f"wt_{j}", bufs=1)
            (nc.sync if j == 0 else nc.gpsimd).dma_start(
                t[:, 0:C], wt_d[j * 128:(j + 1) * 128, 0:C])
            wt_sb.append(t)
        for j in range(2):
            (nc.sync if j == 0 else nc.gpsimd).dma_start(
                wt_sb[j][:, C:], wt_d[j * 128:(j + 1) * 128, C:])
        # ctx pair tiles: [128, 512] = ctxT[2p, jchunk] | ctxT[2p+1, jchunk]
        # pairs 0,1 first (phase-1 g0 critical path), queues interleaved
        ctxp_sb = {}
        qi = 0
        for p in range(B // 2):
            for j in range(2):
                t = ctx_pool.tile([128, 2 * S], f16, name=f"ctx_{p}_{j}", bufs=1)
                for h in range(2):
                    eng = nc.sync if qi % 2 == 0 else nc.gpsimd
                    qi += 1
                    eng.dma_start(t[:, h * S:(h + 1) * S],
                                  ctxT_d[2 * p + h, j * 128:(j + 1) * 128, :])
                ctxp_sb[p, j] = t
        foldL_sb = fl_pool.tile([KF, B * S], f16, name="foldL", bufs=1)
        nc.sync.dma_start(foldL_sb[:], foldL_d[:])

        tmp2p = {}

        def phase1(pg, ps1_pool, copy_engines=("vector",)):  # pg: pair indices
            ce = [0]
            for ch in range(2):  # i-chunk (output partition of tmp2)
                for p in pg:
                    # pair tile, layout (h=b-half, k, z)
                    tmp2p[p, ch] = tmp2_pool.tile([128, 2 * KV * S], f16, name="tmp2")
                for kk in range(KV):
                    ps = {}
                    for p in pg:
                        ps[p] = ps1_pool.tile([128, 2 * S], f32, name="ps1")
                    for j in range(2):  # contraction chunk
                        lhsT = wt_sb[j][:, kk * C + ch * 128: kk * C + ch * 128 + 128]
                        for p in pg:
                            nc.tensor.matmul(
                                ps[p][:], lhsT, ctxp_sb[p, j][:],
                                start=(j == 0), stop=(j == 1),
                            )
                    for p in pg:
                        # one copy per bank: psum (h, z) -> pair tile (h, kk, z)
                        dst = tmp2p[p, ch][:].rearrange("q (h k z) -> q h k z", h=2, k=KV)
                        src_ap = ps[p][:].rearrange("q (h z) -> q h z", h=2)
                        eng = copy_engines[ce[0] % len(copy_engines)]
                        ce[0] += 1
                        if eng == "vector":
                            nc.vector.tensor_copy(dst[:, :, kk, :], src_ap)
                        else:
                            nc.scalar.copy(dst[:, :, kk, :], src_ap)

        def phase2(bg, ps2_pool):
            for b in bg:
                frt = fr_pool.tile([KF, S * KV], f16, name="foldR")
                nc.sync.dma_start(frt[:], foldR_d[b])
                for sc in range(2):
                    hoff = (b % 2) * KV * S
                    lhsT3 = foldL_sb[:, b * S + sc * 128: b * S + sc * 128 + 128]
                    for n in range(4):  # one single-bank psum tile per k-pair
                        pst = ps2_pool.tile([128, 512], f32, name="ps2")
                        for st in range(2):  # contraction chunk over i
                            lhsT = ctxp_sb[b // 2, st][:, (b % 2) * S + sc * 128:
                                                       (b % 2) * S + sc * 128 + 128]
                            nc.tensor.matmul(
                                pst[:], lhsT,
                                tmp2p[b // 2, st][:, hoff + n * 512:hoff + (n + 1) * 512],
                                start=(st == 0), stop=False,
                            )
                        nc.tensor.matmul(
                            pst[:], lhsT3,
                            frt[:, n * 512:(n + 1) * 512],
                            start=False, stop=True,
                        )
                        ot = outs_pool.tile([128, 512], mybir.dt.float16, name="ot")
                        nc.scalar.activation(ot[:], pst[:], TANH)
                        eng = nc.sync if n % 2 == 0 else nc.gpsimd
                        eng.dma_start(
                            out_d[b, sc * 128:(sc + 1) * 128,
                                  2 * n:2 * n + 2].rearrange("s k z -> s (k z)"),
                            ot[:],
                        )

        # 2-pair phase-1 groups amortize weight loads; early phase-2 work is
        # PE fill-in while phase-1 waits on DVE drains (and vice versa later)
        with tc.tile_pool(name="ps1", bufs=4, space="PSUM") as ps1_pool, \
                tc.tile_pool(name="ps2a", bufs=4, space="PSUM") as ps2a:
            phase1([0, 1], ps1_pool, copy_engines=("vector", "scalar"))
            phase2([0], ps2a)
            phase2([1], ps2a)
            phase1([2, 3], ps1_pool, copy_engines=("vector",))
            phase2([2], ps2a)
            phase2([3], ps2a)
        with tc.tile_pool(name="ps2b", bufs=8, space="PSUM") as ps2b:
            for b in range(4, 8):
                phase2([b], ps2b)

    nc.compile()
    return nc


def _install_profile_hook():
    """Register the NTFF profile hook that the image's boot skipped
    (antenv.axon_hooks shim is missing in this container)."""
    import sys as _sys
    import types as _types
    try:
        import antenv
        if "antenv.axon_hooks" not in _sys.modules:
            m = _types.ModuleType("antenv.axon_hooks")
            _h = [None]
            m.set_axon_ntff_profile_hook = lambda h: _h.__setitem__(0, h)
            m.get_axon_ntff_profile_hook = lambda: _h[0]
            _sys.modules["antenv.axon_hooks"] = m
            antenv.axon_hooks = m
        from antenv.axon_hooks import set_axon_ntff_profile_hook, get_axon_ntff_profile_hook
        if get_axon_ntff_profile_hook() is None:
            from trn_agent_boot.trn_boot import _ntff_profile_via_ctypes
            set_axon_ntff_profile_hook(_ntff_profile_via_ctypes("/opt/axon/libaxon_pjrt.so"))
    except Exception:
        pass


def _patch_walrus_ldw_opt():
    """Enable walrus LDWEIGHTS dedup (concourse hardcodes it off). With fp32r
    matmuls walrus emits one LDWEIGHTS per matmul; repeated identical loads
    serialize against the matmul stream (same-row-group loads can't pull
    ahead), costing ~170-300ns per matmul."""
    import os
    if os.environ.get("KERNEL_LDW_OPT", "0") != "1":
        return
    import concourse.bass_utils as bu
    if getattr(bu.run_command, "_ldw_patched", False):
        return
    orig = bu.run_command

    def patched(argv, **kw):
        argv = ["--enable-ldw-opt=true" if a == "--enable-ldw-opt=false" else a
                for a in argv]
        return orig(argv, **kw)

    patched._ldw_patched = True
    bu.run_command = patched


def run(inputs, trace=False, repeats=1):
    """Returns (full_output, BassKernelResults)."""
    from concourse.bass_utils import run_bass_kernel_spmd

    _patch_walrus_ldw_opt()
    if trace:
        _install_profile_hook()
    per_core = _host_prep(**inputs)
    nc = _build_program()
    import os as _os
    _tc = [int(x) for x in _os.environ.get("KERNEL_TRACE_CORES", "0").split(",")]
    times = []
    for r in range(repeats):
        res = run_bass_kernel_spmd(nc, per_core, list(range(NCORES)), trace=trace,
                                   trace_cores=_tc if trace else None)
        if res.exec_time_ns is not None:
            times.append(res.exec_time_ns)
    if times:
        res.all_exec_times_ns = times
    # per-core scratch is (B, S, KV, S) with k-major planes: swap to (B,S,S,KV)
    out = np.concatenate(
        [res.results[c]["out"].astype(np.float32).transpose(0, 1, 3, 2)
         for c in range(NCORES)], axis=3)
    out = np.ascontiguousarray(out)
    return out, res


def kernel(**inputs) -> np.ndarray:
    out, _ = run(inputs, trace=False)
    return out


# revision 29
# speedup vs baseline: 1.1974x; 1.0681x over previous
"""Trainium2 Bass kernel for nn_BilinAndFwdComboVecComp.

Math (B=8, S=256, C=256, V=64):
  final[b,s,z,k] = tanh( sum_ij ctx[b,s,i] ctx[b,z,j] W'[i,j,k] + A[b,z,k] + Bt[b,s,k] )
where
  W'[i,j,k] = W[i,j,k] + (i==j) * linmul_w[k,i]          (folds the `mul` branch)
  A[b,z,k]  = ctx[b] @ (lin1_w+lindiff_w).T + (lin1_b + bias + linmul_b + lindiff_b)
  Bt[b,s,k] = ctx[b] @ (lin2_w-lindiff_w).T + lin2_b     (the `diff` branch is rank-1
                                                          per pair and merges into A/Bt)

Sharding: V split across the 8 cores (8 k-values per core). Each core:
  phase 1: tmp2[i,(k,z)] = sum_j Wt[j,(k,i)]-slices^T @ ctxT[j,z]   (W-stationary,
           batch pairs share each weight load; PSUM drained by DVE/ACT copies)
  phase 2: out[s,(k,z)]  = ctxT[:,s]^T @ tmp2[:,(k,z)] + K=18 fold matmul
           (Bt via fp16-hi/lo delta rows, A via fp16-hi/lo ones rows), tanh on ACT
           (fp16 out), DMA to a (B,S,KV,S) scratch; host transposes/concats.
Matmuls run in fp16 (full PE rate, FWL weight loads overlap the stream; ~11-bit
mantissa matches float32r's effective precision class). Phase-1 single-pair slices
are interleaved through the phase-2 stream so the PE always has surplus work, and
the whole kernel finishes before the package-level power throttle (~80us) engages.
KERNEL_DTYPE=f32r env switches to float32r compute (lower error, ~35% slower).
"""

import numpy as np

B, S, C, V = 8, 256, 256, 64
NCORES = 8
KV = V // NCORES  # k-values per core


def _host_prep(ctx, W, bias, lin1_w, lin1_b, lin2_w, lin2_b,
               linmul_w, linmul_b, lindiff_w, lindiff_b):
    f = np.float32
    ctx = np.asarray(ctx, f)
    Wp = np.array(W, f)
    Wp[np.arange(C), np.arange(C), :] += np.asarray(linmul_w, f).T
    Wt = Wp.transpose(1, 0, 2)  # [j, i, k]

    A = ctx @ (np.asarray(lin1_w, f) + np.asarray(lindiff_w, f)).T \
        + (np.asarray(lin1_b, f) + np.asarray(bias, f)
           + np.asarray(linmul_b, f) + np.asarray(lindiff_b, f))
    Bt = ctx @ (np.asarray(lin2_w, f) - np.asarray(lindiff_w, f)).T + np.asarray(lin2_b, f)

    ctxT = np.ascontiguousarray(ctx.transpose(0, 2, 1))  # [B, C, S]

    # delta in (k, z) layout: row r is 1 over the z-block of plane k==r
    delta = np.zeros((KV, KV * S), f)
    for r in range(KV):
        delta[r, r * S:(r + 1) * S] = 1.0

    per_core = []
    for c in range(NCORES):
        ks = slice(c * KV, (c + 1) * KV)
        # wt layout: [j, kk*C + i]
        wt = np.ascontiguousarray(Wt[:, :, ks].transpose(0, 2, 1).reshape(C, KV * C))
        # fold contraction, K = 2*KV+2 rows: Bt hi/lo via delta rows (exact in
        # fp16 as hi + residual), A hi/lo via ones rows
        Btc = Bt[:, :, ks].transpose(2, 0, 1).reshape(KV, B * S)
        Btc_hi = Btc.astype(np.float16).astype(f)
        Ac = A[:, :, ks].transpose(0, 2, 1).reshape(B, KV * S)
        Ac_hi = Ac.astype(np.float16).astype(f)
        KF = 2 * KV + 2
        foldL = np.empty((KF, B * S), f)
        foldL[:KV] = Btc_hi
        foldL[KV:2 * KV] = Btc - Btc_hi
        foldL[2 * KV:] = 1.0
        foldR = np.empty((B, KF, KV * S), f)
        foldR[:, :KV, :] = delta[None]
        foldR[:, KV:2 * KV, :] = delta[None]
        foldR[:, 2 * KV, :] = Ac_hi
        foldR[:, 2 * KV + 1, :] = Ac - Ac_hi
        per_core.append({"ctxT": ctxT, "wt": wt, "foldL": foldL, "foldR": foldR})
    import os
    if os.environ.get("KERNEL_DTYPE", "f16") == "f16":
        per_core = [{k: v.astype(np.float16) for k, v in m.items()} for m in per_core]
    return per_core


def _build_program():
    import concourse.tile as tile
    import concourse.mybir as mybir
    from concourse import bacc
    from contextlib import ExitStack

    import os
    f32 = mybir.dt.float32
    f16 = mybir.dt.float16
    if os.environ.get("KERNEL_DTYPE", "f16") == "f32r":
        f16 = mybir.dt.float32r  # compute dtype for matmul operands
    TANH = mybir.ActivationFunctionType.Tanh

    nc = bacc.Bacc("TRN2", target_bir_lowering=False, debug=False)
    ctxT_d = nc.dram_tensor("ctxT", [B, C, S], f16, kind="ExternalInput").ap()
    wt_d = nc.dram_tensor("wt", [C, KV * C], f16, kind="ExternalInput").ap()
    KF = 2 * KV + 2
    foldL_d = nc.dram_tensor("foldL", [KF, B * S], f16, kind="ExternalInput").ap()
    foldR_d = nc.dram_tensor("foldR", [B, KF, S * KV], f16, kind="ExternalInput").ap()
    # out scratch is (k, z)-ordered; the host transposes back to (z, k)
    out_d = nc.dram_tensor("out", [B, S, KV, S], mybir.dt.float16, kind="ExternalOutput").ap()

    with tile.TileContext(nc) as tc, ExitStack() as es:
        ctx_pool = es.enter_context(tc.tile_pool(name="ctxp", bufs=8))
        wt_pool = es.enter_context(tc.tile_pool(name="wtp", bufs=2))
        fl_pool = es.enter_context(tc.tile_pool(name="flp", bufs=1))
        fr_pool = es.enter_context(tc.tile_pool(name="frp", bufs=3))
        tmp2_pool = es.enter_context(tc.tile_pool(name="tmp2p", bufs=7))
        outs_pool = es.enter_context(tc.tile_pool(name="outsp", bufs=4))


        # weights kk=0 slices first (split across two DMA sequencers) so the
        # PE starts ASAP
        wt_sb = []
        for j in range(2):
            t = wt_pool.tile([128, KV * C], f16, name=f"wt_{j}", bufs=1)
            (nc.sync if j == 0 else nc.gpsimd).dma_start(
                t[:, 0:C], wt_d[j * 128:(j + 1) * 128, 0:C])
            wt_sb.append(t)
        for j in range(2):
            (nc.sync if j == 0 else nc.gpsimd).dma_start(
                wt_sb[j][:, C:], wt_d[j * 128:(j + 1) * 128, C:])
        # ctx pair tiles: [128, 512] = ctxT[2p, jchunk] | ctxT[2p+1, jchunk]
        # pairs 0,1 first (phase-1 g0 critical path), queues interleaved
        ctxp_sb = {}
        qi = 0
        for p in range(B // 2):
            for j in range(2):
                t = ctx_pool.tile([128, 2 * S], f16, name=f"ctx_{p}_{j}", bufs=1)
                for h in range(2):
                    eng = nc.sync if qi % 2 == 0 else nc.gpsimd
                    qi += 1
                    eng.dma_start(t[:, h * S:(h + 1) * S],
                                  ctxT_d[2 * p + h, j * 128:(j + 1) * 128, :])
                ctxp_sb[p, j] = t
        foldL_sb = fl_pool.tile([KF, B * S], f16, name="foldL", bufs=1)
        nc.sync.dma_start(foldL_sb[:], foldL_d[:])

        tmp2p = {}

        def phase1(pg, ps1_pool, copy_engines=("vector",)):  # pg: pair indices
            ce = [0]
            for ch in range(2):  # i-chunk (output partition of tmp2)
                for p in pg:
                    # pair tile, layout (h=b-half, k, z)
                    tmp2p[p, ch] = tmp2_pool.tile([128, 2 * KV * S], f16, name="tmp2")
                for kk in range(KV):
                    ps = {}
                    for p in pg:
                        ps[p] = ps1_pool.tile([128, 2 * S], f32, name="ps1")
                    for j in range(2):  # contraction chunk
                        lhsT = wt_sb[j][:, kk * C + ch * 128: kk * C + ch * 128 + 128]
                        for p in pg:
                            nc.tensor.matmul(
                                ps[p][:], lhsT, ctxp_sb[p, j][:],
                                start=(j == 0), stop=(j == 1),
                            )
                    for p in pg:
                        # one copy per bank: psum (h, z) -> pair tile (h, kk, z)
                        dst = tmp2p[p, ch][:].rearrange("q (h k z) -> q h k z", h=2, k=KV)
                        src_ap = ps[p][:].rearrange("q (h z) -> q h z", h=2)
                        eng = copy_engines[ce[0] % len(copy_engines)]
                        ce[0] += 1
                        if eng == "vector":
                            nc.vector.tensor_copy(dst[:, :, kk, :], src_ap)
                        else:
                            nc.scalar.copy(dst[:, :, kk, :], src_ap)

        def phase2(bg, ps2_pool):
            for b in bg:
                frt = fr_pool.tile([KF, S * KV], f16, name="foldR")
                nc.sync.dma_start(frt[:], foldR_d[b])
                for sc in range(2):
                    hoff = (b % 2) * KV * S
                    lhsT3 = foldL_sb[:, b * S + sc * 128: b * S + sc * 128 + 128]
                    for t in range(2):  # double-bank psum tiles, 2 n-chunks each
                        pst = ps2_pool.tile([128, 1024], f32, name="ps2")
                        n0 = 2 * t
                        for st in range(2):  # contraction chunk; one LDW per 2 MMs
                            lhsT = ctxp_sb[b // 2, st][:, (b % 2) * S + sc * 128:
                                                       (b % 2) * S + sc * 128 + 128]
                            for n in (n0, n0 + 1):
                                nc.tensor.matmul(
                                    pst[:, (n % 2) * 512:(n % 2) * 512 + 512], lhsT,
                                    tmp2p[b // 2, st][:, hoff + n * 512:hoff + (n + 1) * 512],
                                    start=(st == 0), stop=False,
                                )
                        for n in (n0, n0 + 1):
                            nc.tensor.matmul(
                                pst[:, (n % 2) * 512:(n % 2) * 512 + 512], lhsT3,
                                frt[:, n * 512:(n + 1) * 512],
                                start=False, stop=True,
                            )
                        ot = outs_pool.tile([128, 1024], mybir.dt.float16, name="ot")
                        nc.scalar.activation(ot[:], pst[:], TANH)
                        for hd in range(2):
                            eng = nc.sync if hd == 0 else nc.gpsimd
                            eng.dma_start(
                                out_d[b, sc * 128:(sc + 1) * 128,
                                      4 * t + 2 * hd:4 * t + 2 * hd + 2]
                                .rearrange("s k z -> s (k z)"),
                                ot[:, hd * 512:(hd + 1) * 512],
                            )

        # single-pair phase-1 slices interleaved through the whole phase-2
        # stream: the PE always has surplus work while DVE copies or ACT tanh
        # drains pace their own phase
        ps1_pool = es.enter_context(tc.tile_pool(name="ps1", bufs=4, space="PSUM"))
        ps2_pool = es.enter_context(tc.tile_pool(name="ps2", bufs=2, space="PSUM"))
        phase1([0], ps1_pool, copy_engines=("vector", "scalar"))
        phase2([0], ps2_pool)
        phase1([1], ps1_pool, copy_engines=("vector",))
        phase2([1], ps2_pool)
        phase2([2], ps2_pool)
        phase1([2], ps1_pool, copy_engines=("vector",))
        phase2([3], ps2_pool)
        phase2([4], ps2_pool)
        phase1([3], ps1_pool, copy_engines=("vector",))
        phase2([5], ps2_pool)
        phase2([6], ps2_pool)
        phase2([7], ps2_pool)

    nc.compile()
    return nc


def _install_profile_hook():
    """Register the NTFF profile hook that the image's boot skipped
    (antenv.axon_hooks shim is missing in this container)."""
    import sys as _sys
    import types as _types
    try:
        import antenv
        if "antenv.axon_hooks" not in _sys.modules:
            m = _types.ModuleType("antenv.axon_hooks")
            _h = [None]
            m.set_axon_ntff_profile_hook = lambda h: _h.__setitem__(0, h)
            m.get_axon_ntff_profile_hook = lambda: _h[0]
            _sys.modules["antenv.axon_hooks"] = m
            antenv.axon_hooks = m
        from antenv.axon_hooks import set_axon_ntff_profile_hook, get_axon_ntff_profile_hook
        if get_axon_ntff_profile_hook() is None:
            from trn_agent_boot.trn_boot import _ntff_profile_via_ctypes
            set_axon_ntff_profile_hook(_ntff_profile_via_ctypes("/opt/axon/libaxon_pjrt.so"))
    except Exception:
        pass


def _patch_walrus_ldw_opt():
    """Enable walrus LDWEIGHTS dedup (concourse hardcodes it off). With fp32r
    matmuls walrus emits one LDWEIGHTS per matmul; repeated identical loads
    serialize against the matmul stream (same-row-group loads can't pull
    ahead), costing ~170-300ns per matmul."""
    import os
    if os.environ.get("KERNEL_LDW_OPT", "0") != "1":
        return
    import concourse.bass_utils as bu
    if getattr(bu.run_command, "_ldw_patched", False):
        return
    orig = bu.run_command

    def patched(argv, **kw):
        argv = ["--enable-ldw-opt=true" if a == "--enable-ldw-opt=false" else a
                for a in argv]
        return orig(argv, **kw)

    patched._ldw_patched = True
    bu.run_command = patched


def run(inputs, trace=False, repeats=1):
    """Returns (full_output, BassKernelResults)."""
    from concourse.bass_utils import run_bass_kernel_spmd

    _patch_walrus_ldw_opt()
    if trace:
        _install_profile_hook()
    per_core = _host_prep(**inputs)
    nc = _build_program()
    import os as _os
    _tc = [int(x) for x in _os.environ.get("KERNEL_TRACE_CORES", "0").split(",")]
    times = []
    for r in range(repeats):
        res = run_bass_kernel_spmd(nc, per_core, list(range(NCORES)), trace=trace,
                                   trace_cores=_tc if trace else None)
        if res.exec_time_ns is not None:
            times.append(res.exec_time_ns)
    if times:
        res.all_exec_times_ns = times
    # per-core scratch is (B, S, KV, S) with k-major planes: swap to (B,S,S,KV)
    out = np.concatenate(
        [res.results[c]["out"].astype(np.float32).transpose(0, 1, 3, 2)
         for c in range(NCORES)], axis=3)
    out = np.ascontiguousarray(out)
    return out, res


def kernel(**inputs) -> np.ndarray:
    out, _ = run(inputs, trace=False)
    return out


# revision 33
# speedup vs baseline: 1.2013x; 1.0033x over previous
"""Trainium2 Bass kernel for nn_BilinAndFwdComboVecComp.

Math (B=8, S=256, C=256, V=64):
  final[b,s,z,k] = tanh( sum_ij ctx[b,s,i] ctx[b,z,j] W'[i,j,k] + A[b,z,k] + Bt[b,s,k] )
where
  W'[i,j,k] = W[i,j,k] + (i==j) * linmul_w[k,i]          (folds the `mul` branch)
  A[b,z,k]  = ctx[b] @ (lin1_w+lindiff_w).T + (lin1_b + bias + linmul_b + lindiff_b)
  Bt[b,s,k] = ctx[b] @ (lin2_w-lindiff_w).T + lin2_b     (the `diff` branch is rank-1
                                                          per pair and merges into A/Bt)

Sharding: V split across the 8 cores (8 k-values per core). Each core:
  phase 1: tmp2[i,(k,z)] = sum_j Wt[j,(k,i)]-slices^T @ ctxT[j,z]   (W-stationary,
           batch pairs share each weight load; PSUM drained by DVE/ACT copies)
  phase 2: out[s,(k,z)]  = ctxT[:,s]^T @ tmp2[:,(k,z)] + K=18 fold matmul
           (Bt via fp16-hi/lo delta rows, A via fp16-hi/lo ones rows), tanh on ACT
           (fp16 out), DMA to a (B,S,KV,S) scratch; host transposes/concats.
Matmuls run in fp16 (full PE rate, FWL weight loads overlap the stream; ~11-bit
mantissa matches float32r's effective precision class). Phase-1 single-pair slices
are interleaved through the phase-2 stream so the PE always has surplus work, and
the whole kernel finishes before the package-level power throttle (~80us) engages.
KERNEL_DTYPE=f32r env switches to float32r compute (lower error, ~35% slower).
"""

import numpy as np

B, S, C, V = 8, 256, 256, 64
NCORES = 8
KV = V // NCORES  # k-values per core


def _host_prep(ctx, W, bias, lin1_w, lin1_b, lin2_w, lin2_b,
               linmul_w, linmul_b, lindiff_w, lindiff_b):
    f = np.float32
    ctx = np.asarray(ctx, f)
    Wp = np.array(W, f)
    Wp[np.arange(C), np.arange(C), :] += np.asarray(linmul_w, f).T
    Wt = Wp.transpose(1, 0, 2)  # [j, i, k]

    A = ctx @ (np.asarray(lin1_w, f) + np.asarray(lindiff_w, f)).T \
        + (np.asarray(lin1_b, f) + np.asarray(bias, f)
           + np.asarray(linmul_b, f) + np.asarray(lindiff_b, f))
    Bt = ctx @ (np.asarray(lin2_w, f) - np.asarray(lindiff_w, f)).T + np.asarray(lin2_b, f)

    ctxT = np.ascontiguousarray(ctx.transpose(0, 2, 1))  # [B, C, S]

    # delta in (k, z) layout: row r is 1 over the z-block of plane k==r
    delta = np.zeros((KV, KV * S), f)
    for r in range(KV):
        delta[r, r * S:(r + 1) * S] = 1.0

    per_core = []
    for c in range(NCORES):
        ks = slice(c * KV, (c + 1) * KV)
        # wt layout: [j, kk*C + i]
        wt = np.ascontiguousarray(Wt[:, :, ks].transpose(0, 2, 1).reshape(C, KV * C))
        # fold contraction, K = 2*KV+2 rows: Bt hi/lo via delta rows (exact in
        # fp16 as hi + residual), A hi/lo via ones rows
        Btc = Bt[:, :, ks].transpose(2, 0, 1).reshape(KV, B * S)
        Btc_hi = Btc.astype(np.float16).astype(f)
        Ac = A[:, :, ks].transpose(0, 2, 1).reshape(B, KV * S)
        Ac_hi = Ac.astype(np.float16).astype(f)
        KF = 2 * KV + 2
        foldL = np.empty((KF, B * S), f)
        foldL[:KV] = Btc_hi
        foldL[KV:2 * KV] = Btc - Btc_hi
        foldL[2 * KV:] = 1.0
        foldR = np.empty((B, KF, KV * S), f)
        foldR[:, :KV, :] = delta[None]
        foldR[:, KV:2 * KV, :] = delta[None]
        foldR[:, 2 * KV, :] = Ac_hi
        foldR[:, 2 * KV + 1, :] = Ac - Ac_hi
        per_core.append({"ctxT": ctxT, "wt": wt, "foldL": foldL, "foldR": foldR})
    import os
    if os.environ.get("KERNEL_DTYPE", "f16") == "f16":
        per_core = [{k: v.astype(np.float16) for k, v in m.items()} for m in per_core]
    return per_core


def _build_program():
    import concourse.tile as tile
    import concourse.mybir as mybir
    from concourse import bacc
    from contextlib import ExitStack

    import os
    f32 = mybir.dt.float32
    f16 = mybir.dt.float16
    if os.environ.get("KERNEL_DTYPE", "f16") == "f32r":
        f16 = mybir.dt.float32r  # compute dtype for matmul operands
    TANH = mybir.ActivationFunctionType.Tanh

    nc = bacc.Bacc("TRN2", target_bir_lowering=False, debug=False)
    ctxT_d = nc.dram_tensor("ctxT", [B, C, S], f16, kind="ExternalInput").ap()
    wt_d = nc.dram_tensor("wt", [C, KV * C], f16, kind="ExternalInput").ap()
    KF = 2 * KV + 2
    foldL_d = nc.dram_tensor("foldL", [KF, B * S], f16, kind="ExternalInput").ap()
    foldR_d = nc.dram_tensor("foldR", [B, KF, S * KV], f16, kind="ExternalInput").ap()
    # out scratch is (k, z)-ordered; the host transposes back to (z, k)
    out_d = nc.dram_tensor("out", [B, S, KV, S], mybir.dt.float16, kind="ExternalOutput").ap()

    with tile.TileContext(nc) as tc, ExitStack() as es:
        ctx_pool = es.enter_context(tc.tile_pool(name="ctxp", bufs=8))
        wt_pool = es.enter_context(tc.tile_pool(name="wtp", bufs=2))
        fl_pool = es.enter_context(tc.tile_pool(name="flp", bufs=1))
        fr_pool = es.enter_context(tc.tile_pool(name="frp", bufs=3))
        tmp2_pool = es.enter_context(tc.tile_pool(name="tmp2p", bufs=8))
        outs_pool = es.enter_context(tc.tile_pool(name="outsp", bufs=6))


        # weights kk=0 slices first (split across two DMA sequencers) so the
        # PE starts ASAP
        wt_sb = []
        for j in range(2):
            t = wt_pool.tile([128, KV * C], f16, name=f"wt_{j}", bufs=1)
            (nc.sync if j == 0 else nc.gpsimd).dma_start(
                t[:, 0:C], wt_d[j * 128:(j + 1) * 128, 0:C])
            wt_sb.append(t)
        for j in range(2):
            (nc.sync if j == 0 else nc.gpsimd).dma_start(
                wt_sb[j][:, C:], wt_d[j * 128:(j + 1) * 128, C:])
        # ctx pair tiles: [128, 512] = ctxT[2p, jchunk] | ctxT[2p+1, jchunk]
        # pairs 0,1 first (phase-1 g0 critical path), queues interleaved
        ctxp_sb = {}
        qi = 0
        for p in range(B // 2):
            for j in range(2):
                t = ctx_pool.tile([128, 2 * S], f16, name=f"ctx_{p}_{j}", bufs=1)
                eng = nc.sync if qi % 2 == 0 else nc.gpsimd
                qi += 1
                eng.dma_start(
                    t[:].rearrange("c (h z) -> c h z", h=2),
                    ctxT_d[2 * p:2 * p + 2, j * 128:(j + 1) * 128, :]
                    .rearrange("h c z -> c h z"),
                )
                ctxp_sb[p, j] = t
        foldL_sb = fl_pool.tile([KF, B * S], f16, name="foldL", bufs=1)
        nc.sync.dma_start(foldL_sb[:], foldL_d[:])

        tmp2p = {}

        def phase1(pg, ps1_pool, copy_engines=("vector",), chs=(0, 1)):
            ce = [0]
            for ch in chs:  # i-chunk (output partition of tmp2)
                for p in pg:
                    # pair tile, layout (h=b-half, k, z)
                    tmp2p[p, ch] = tmp2_pool.tile([128, 2 * KV * S], f16, name="tmp2")
                for kk in range(KV):
                    ps = {}
                    for p in pg:
                        ps[p] = ps1_pool.tile([128, 2 * S], f32, name="ps1")
                    for j in range(2):  # contraction chunk
                        lhsT = wt_sb[j][:, kk * C + ch * 128: kk * C + ch * 128 + 128]
                        for p in pg:
                            nc.tensor.matmul(
                                ps[p][:], lhsT, ctxp_sb[p, j][:],
                                start=(j == 0), stop=(j == 1),
                            )
                    for p in pg:
                        # one copy per bank: psum (h, z) -> pair tile (h, kk, z)
                        dst = tmp2p[p, ch][:].rearrange("q (h k z) -> q h k z", h=2, k=KV)
                        src_ap = ps[p][:].rearrange("q (h z) -> q h z", h=2)
                        eng = copy_engines[ce[0] % len(copy_engines)]
                        ce[0] += 1
                        if eng == "vector":
                            nc.vector.tensor_copy(dst[:, :, kk, :], src_ap)
                        else:
                            nc.scalar.copy(dst[:, :, kk, :], src_ap)

        def phase2(bg, ps2_pool):
            for b in bg:
                frt = fr_pool.tile([KF, S * KV], f16, name="foldR")
                nc.sync.dma_start(frt[:], foldR_d[b])
                for sc in range(2):
                    hoff = (b % 2) * KV * S
                    lhsT3 = foldL_sb[:, b * S + sc * 128: b * S + sc * 128 + 128]
                    for t in range(2):  # double-bank psum tiles, 2 n-chunks each
                        pst = ps2_pool.tile([128, 1024], f32, name="ps2")
                        n0 = 2 * t
                        for st in range(2):  # contraction chunk; one LDW per 2 MMs
                            lhsT = ctxp_sb[b // 2, st][:, (b % 2) * S + sc * 128:
                                                       (b % 2) * S + sc * 128 + 128]
                            for n in (n0, n0 + 1):
                                nc.tensor.matmul(
                                    pst[:, (n % 2) * 512:(n % 2) * 512 + 512], lhsT,
                                    tmp2p[b // 2, st][:, hoff + n * 512:hoff + (n + 1) * 512],
                                    start=(st == 0), stop=False,
                                )
                        for n in (n0, n0 + 1):
                            nc.tensor.matmul(
                                pst[:, (n % 2) * 512:(n % 2) * 512 + 512], lhsT3,
                                frt[:, n * 512:(n + 1) * 512],
                                start=False, stop=True,
                            )
                        ot = outs_pool.tile([128, 1024], mybir.dt.float16, name="ot")
                        nc.scalar.activation(ot[:], pst[:], TANH)
                        for hd in range(2):
                            eng = nc.sync if hd == 0 else nc.gpsimd
                            eng.dma_start(
                                out_d[b, sc * 128:(sc + 1) * 128,
                                      4 * t + 2 * hd:4 * t + 2 * hd + 2]
                                .rearrange("s k z -> s (k z)"),
                                ot[:, hd * 512:(hd + 1) * 512],
                            )

        # single-pair phase-1 slices interleaved through the whole phase-2
        # stream: the PE always has surplus work while DVE copies or ACT tanh
        # drains pace their own phase
        ps1_pool = es.enter_context(tc.tile_pool(name="ps1", bufs=4, space="PSUM"))
        ps2_pool = es.enter_context(tc.tile_pool(name="ps2", bufs=2, space="PSUM"))
        mix = ("vector", "vector", "vector", "scalar")
        phase1([0], ps1_pool, copy_engines=("vector", "scalar"))
        phase2([0], ps2_pool)
        phase1([1], ps1_pool, copy_engines=mix)
        phase2([1], ps2_pool)
        phase2([2], ps2_pool)
        phase1([2], ps1_pool, copy_engines=mix)
        phase2([3], ps2_pool)
        phase1([3], ps1_pool, copy_engines=mix, chs=(0,))
        phase2([4], ps2_pool)
        phase1([3], ps1_pool, copy_engines=mix, chs=(1,))
        phase2([5], ps2_pool)
        phase2([6], ps2_pool)
        phase2([7], ps2_pool)

    nc.compile()
    return nc


def _install_profile_hook():
    """Register the NTFF profile hook that the image's boot skipped
    (antenv.axon_hooks shim is missing in this container)."""
    import sys as _sys
    import types as _types
    try:
        import antenv
        if "antenv.axon_hooks" not in _sys.modules:
            m = _types.ModuleType("antenv.axon_hooks")
            _h = [None]
            m.set_axon_ntff_profile_hook = lambda h: _h.__setitem__(0, h)
            m.get_axon_ntff_profile_hook = lambda: _h[0]
            _sys.modules["antenv.axon_hooks"] = m
            antenv.axon_hooks = m
        from antenv.axon_hooks import set_axon_ntff_profile_hook, get_axon_ntff_profile_hook
        if get_axon_ntff_profile_hook() is None:
            from trn_agent_boot.trn_boot import _ntff_profile_via_ctypes
            set_axon_ntff_profile_hook(_ntff_profile_via_ctypes("/opt/axon/libaxon_pjrt.so"))
    except Exception:
        pass


def _patch_walrus_ldw_opt():
    """Enable walrus LDWEIGHTS dedup (concourse hardcodes it off). With fp32r
    matmuls walrus emits one LDWEIGHTS per matmul; repeated identical loads
    serialize against the matmul stream (same-row-group loads can't pull
    ahead), costing ~170-300ns per matmul."""
    import os
    if os.environ.get("KERNEL_LDW_OPT", "0") != "1":
        return
    import concourse.bass_utils as bu
    if getattr(bu.run_command, "_ldw_patched", False):
        return
    orig = bu.run_command

    def patched(argv, **kw):
        argv = ["--enable-ldw-opt=true" if a == "--enable-ldw-opt=false" else a
                for a in argv]
        return orig(argv, **kw)

    patched._ldw_patched = True
    bu.run_command = patched


def run(inputs, trace=False, repeats=1):
    """Returns (full_output, BassKernelResults)."""
    from concourse.bass_utils import run_bass_kernel_spmd

    _patch_walrus_ldw_opt()
    if trace:
        _install_profile_hook()
    per_core = _host_prep(**inputs)
    nc = _build_program()
    import os as _os
    _tc = [int(x) for x in _os.environ.get("KERNEL_TRACE_CORES", "0").split(",")]
    times = []
    for r in range(repeats):
        res = run_bass_kernel_spmd(nc, per_core, list(range(NCORES)), trace=trace,
                                   trace_cores=_tc if trace else None)
        if res.exec_time_ns is not None:
            times.append(res.exec_time_ns)
    if times:
        res.all_exec_times_ns = times
    # per-core scratch is (B, S, KV, S) with k-major planes: swap to (B,S,S,KV)
    out = np.concatenate(
        [res.results[c]["out"].astype(np.float32).transpose(0, 1, 3, 2)
         for c in range(NCORES)], axis=3)
    out = np.ascontiguousarray(out)
    return out, res


def kernel(**inputs) -> np.ndarray:
    out, _ = run(inputs, trace=False)
    return out


# revision 35
# speedup vs baseline: 1.2052x; 1.0032x over previous
"""Trainium2 Bass kernel for nn_BilinAndFwdComboVecComp.

Math (B=8, S=256, C=256, V=64):
  final[b,s,z,k] = tanh( sum_ij ctx[b,s,i] ctx[b,z,j] W'[i,j,k] + A[b,z,k] + Bt[b,s,k] )
where
  W'[i,j,k] = W[i,j,k] + (i==j) * linmul_w[k,i]          (folds the `mul` branch)
  A[b,z,k]  = ctx[b] @ (lin1_w+lindiff_w).T + (lin1_b + bias + linmul_b + lindiff_b)
  Bt[b,s,k] = ctx[b] @ (lin2_w-lindiff_w).T + lin2_b     (the `diff` branch is rank-1
                                                          per pair and merges into A/Bt)

Sharding: V split across the 8 cores (8 k-values per core). Each core:
  phase 1: tmp2[i,(k,z)] = sum_j Wt[j,(k,i)]-slices^T @ ctxT[j,z]   (W-stationary,
           batch pairs share each weight load; PSUM drained by DVE/ACT copies)
  phase 2: out[s,(k,z)]  = ctxT[:,s]^T @ tmp2[:,(k,z)] + K=18 fold matmul
           (Bt via fp16-hi/lo delta rows, A via fp16-hi/lo ones rows), tanh on ACT
           (fp16 out), DMA to a (B,S,KV,S) scratch; host transposes/concats.
Matmuls run in fp16 (full PE rate, FWL weight loads overlap the stream; ~11-bit
mantissa matches float32r's effective precision class). Phase-1 single-pair slices
are interleaved through the phase-2 stream so the PE always has surplus work, and
the whole kernel finishes before the package-level power throttle (~80us) engages.
KERNEL_DTYPE=f32r env switches to float32r compute (lower error, ~35% slower).
"""

import numpy as np

B, S, C, V = 8, 256, 256, 64
NCORES = 8
KV = V // NCORES  # k-values per core


def _host_prep(ctx, W, bias, lin1_w, lin1_b, lin2_w, lin2_b,
               linmul_w, linmul_b, lindiff_w, lindiff_b):
    f = np.float32
    ctx = np.asarray(ctx, f)
    Wp = np.array(W, f)
    Wp[np.arange(C), np.arange(C), :] += np.asarray(linmul_w, f).T
    Wt = Wp.transpose(1, 0, 2)  # [j, i, k]

    A = ctx @ (np.asarray(lin1_w, f) + np.asarray(lindiff_w, f)).T \
        + (np.asarray(lin1_b, f) + np.asarray(bias, f)
           + np.asarray(linmul_b, f) + np.asarray(lindiff_b, f))
    Bt = ctx @ (np.asarray(lin2_w, f) - np.asarray(lindiff_w, f)).T + np.asarray(lin2_b, f)

    ctxT = np.ascontiguousarray(ctx.transpose(0, 2, 1))  # [B, C, S]

    # delta in (k, z) layout: row r is 1 over the z-block of plane k==r
    delta = np.zeros((KV, KV * S), f)
    for r in range(KV):
        delta[r, r * S:(r + 1) * S] = 1.0

    per_core = []
    for c in range(NCORES):
        ks = slice(c * KV, (c + 1) * KV)
        # wt layout: [j, kk*C + i]
        wt = np.ascontiguousarray(Wt[:, :, ks].transpose(0, 2, 1).reshape(C, KV * C))
        # fold contraction, K = 2*KV+2 rows: Bt hi/lo via delta rows (exact in
        # fp16 as hi + residual), A hi/lo via ones rows
        Btc = Bt[:, :, ks].transpose(2, 0, 1).reshape(KV, B * S)
        Btc_hi = Btc.astype(np.float16).astype(f)
        Ac = A[:, :, ks].transpose(0, 2, 1).reshape(B, KV * S)
        Ac_hi = Ac.astype(np.float16).astype(f)
        KF = 2 * KV + 2
        foldL = np.empty((KF, B * S), f)
        foldL[:KV] = Btc_hi
        foldL[KV:2 * KV] = Btc - Btc_hi
        foldL[2 * KV:] = 1.0
        foldR = np.empty((B, KF, KV * S), f)
        foldR[:, :KV, :] = delta[None]
        foldR[:, KV:2 * KV, :] = delta[None]
        foldR[:, 2 * KV, :] = Ac_hi
        foldR[:, 2 * KV + 1, :] = Ac - Ac_hi
        per_core.append({"ctxT": ctxT, "wt": wt, "foldL": foldL, "foldR": foldR})
    import os
    if os.environ.get("KERNEL_DTYPE", "f16") == "f16":
        per_core = [{k: v.astype(np.float16) for k, v in m.items()} for m in per_core]
    return per_core


def _build_program():
    import concourse.tile as tile
    import concourse.mybir as mybir
    from concourse import bacc
    from contextlib import ExitStack

    import os
    f32 = mybir.dt.float32
    f16 = mybir.dt.float16
    if os.environ.get("KERNEL_DTYPE", "f16") == "f32r":
        f16 = mybir.dt.float32r  # compute dtype for matmul operands
    TANH = mybir.ActivationFunctionType.Tanh

    nc = bacc.Bacc("TRN2", target_bir_lowering=False, debug=False)
    ctxT_d = nc.dram_tensor("ctxT", [B, C, S], f16, kind="ExternalInput").ap()
    wt_d = nc.dram_tensor("wt", [C, KV * C], f16, kind="ExternalInput").ap()
    KF = 2 * KV + 2
    foldL_d = nc.dram_tensor("foldL", [KF, B * S], f16, kind="ExternalInput").ap()
    foldR_d = nc.dram_tensor("foldR", [B, KF, S * KV], f16, kind="ExternalInput").ap()
    # out scratch is (k, z)-ordered; the host transposes back to (z, k)
    out_d = nc.dram_tensor("out", [B, S, KV, S], mybir.dt.float16, kind="ExternalOutput").ap()

    with tile.TileContext(nc) as tc, ExitStack() as es:
        ctx_pool = es.enter_context(tc.tile_pool(name="ctxp", bufs=8))
        wt_pool = es.enter_context(tc.tile_pool(name="wtp", bufs=2))
        fl_pool = es.enter_context(tc.tile_pool(name="flp", bufs=1))
        fr_pool = es.enter_context(tc.tile_pool(name="frp", bufs=3))
        tmp2_pool = es.enter_context(tc.tile_pool(name="tmp2p", bufs=8))
        outs_pool = es.enter_context(tc.tile_pool(name="outsp", bufs=6))


        # weights kk=0 slices first (split across two DMA sequencers) so the
        # PE starts ASAP
        wt_sb = []
        for j in range(2):
            t = wt_pool.tile([128, KV * C], f16, name=f"wt_{j}", bufs=1)
            (nc.sync if j == 0 else nc.gpsimd).dma_start(
                t[:, 0:C], wt_d[j * 128:(j + 1) * 128, 0:C])
            wt_sb.append(t)
        for j in range(2):
            (nc.sync if j == 0 else nc.gpsimd).dma_start(
                wt_sb[j][:, C:], wt_d[j * 128:(j + 1) * 128, C:])
        # ctx pair tiles: [128, 512] = ctxT[2p, jchunk] | ctxT[2p+1, jchunk]
        # pairs 0,1 first (phase-1 g0 critical path), queues interleaved
        ctxp_sb = {}
        qi = 0
        for p in range(B // 2):
            for j in range(2):
                t = ctx_pool.tile([128, 2 * S], f16, name=f"ctx_{p}_{j}", bufs=1)
                eng = nc.sync if qi % 2 == 0 else nc.gpsimd
                qi += 1
                eng.dma_start(
                    t[:].rearrange("c (h z) -> c h z", h=2),
                    ctxT_d[2 * p:2 * p + 2, j * 128:(j + 1) * 128, :]
                    .rearrange("h c z -> c h z"),
                )
                ctxp_sb[p, j] = t
        foldL_sb = fl_pool.tile([KF, B * S], f16, name="foldL", bufs=1)
        nc.sync.dma_start(foldL_sb[:], foldL_d[:])

        tmp2p = {}

        def phase1(pg, ps1_pool, copy_engines=("vector",), chs=(0, 1)):
            ce = [0]
            for ch in chs:  # i-chunk (output partition of tmp2)
                for p in pg:
                    # pair tile, layout (h=b-half, k, z)
                    tmp2p[p, ch] = tmp2_pool.tile([128, 2 * KV * S], f16, name="tmp2")
                for kk in range(KV):
                    ps = {}
                    for p in pg:
                        ps[p] = ps1_pool.tile([128, 2 * S], f32, name="ps1")
                    for j in range(2):  # contraction chunk
                        lhsT = wt_sb[j][:, kk * C + ch * 128: kk * C + ch * 128 + 128]
                        for p in pg:
                            nc.tensor.matmul(
                                ps[p][:], lhsT, ctxp_sb[p, j][:],
                                start=(j == 0), stop=(j == 1),
                            )
                    for p in pg:
                        # one copy per bank: psum (h, z) -> pair tile (h, kk, z)
                        dst = tmp2p[p, ch][:].rearrange("q (h k z) -> q h k z", h=2, k=KV)
                        src_ap = ps[p][:].rearrange("q (h z) -> q h z", h=2)
                        eng = copy_engines[ce[0] % len(copy_engines)]
                        ce[0] += 1
                        if eng == "vector":
                            nc.vector.tensor_copy(dst[:, :, kk, :], src_ap)
                        else:
                            nc.scalar.copy(dst[:, :, kk, :], src_ap)

        def phase2(bg, ps2_pool):
            for b in bg:
                frt = fr_pool.tile([KF, S * KV], f16, name="foldR")
                nc.sync.dma_start(frt[:], foldR_d[b])
                for sc in range(2):
                    hoff = (b % 2) * KV * S
                    lhsT3 = foldL_sb[:, b * S + sc * 128: b * S + sc * 128 + 128]
                    for t in range(2):  # double-bank psum tiles, 2 n-chunks each
                        pst = ps2_pool.tile([128, 1024], f32, name="ps2")
                        n0 = 2 * t
                        for st in range(2):  # contraction chunk; one LDW per 2 MMs
                            lhsT = ctxp_sb[b // 2, st][:, (b % 2) * S + sc * 128:
                                                       (b % 2) * S + sc * 128 + 128]
                            for n in (n0, n0 + 1):
                                nc.tensor.matmul(
                                    pst[:, (n % 2) * 512:(n % 2) * 512 + 512], lhsT,
                                    tmp2p[b // 2, st][:, hoff + n * 512:hoff + (n + 1) * 512],
                                    start=(st == 0), stop=False,
                                )
                        for n in (n0, n0 + 1):
                            nc.tensor.matmul(
                                pst[:, (n % 2) * 512:(n % 2) * 512 + 512], lhsT3,
                                frt[:, n * 512:(n + 1) * 512],
                                start=False, stop=True,
                            )
                        ot = outs_pool.tile([128, 1024], mybir.dt.float16, name="ot")
                        nc.scalar.activation(ot[:], pst[:], TANH)
                        for hd in range(2):
                            eng = nc.sync if hd == 0 else nc.gpsimd
                            eng.dma_start(
                                out_d[b, sc * 128:(sc + 1) * 128,
                                      4 * t + 2 * hd:4 * t + 2 * hd + 2]
                                .rearrange("s k z -> s (k z)"),
                                ot[:, hd * 512:(hd + 1) * 512],
                            )

        # single-pair phase-1 slices interleaved through the whole phase-2
        # stream: the PE always has surplus work while DVE copies or ACT tanh
        # drains pace their own phase
        ps1_pool = es.enter_context(tc.tile_pool(name="ps1", bufs=4, space="PSUM"))
        ps2_pool = es.enter_context(tc.tile_pool(name="ps2", bufs=2, space="PSUM"))
        mix = ("vector", "vector", "vector", "scalar")
        phase1([0], ps1_pool, copy_engines=("vector", "scalar"))
        phase2([0], ps2_pool)
        phase1([1], ps1_pool, copy_engines=mix)
        phase2([1], ps2_pool)
        phase2([2], ps2_pool)
        phase1([2], ps1_pool, copy_engines=mix)
        phase2([3], ps2_pool)
        phase1([3], ps1_pool, copy_engines=mix, chs=(0,))
        phase2([4], ps2_pool)
        phase1([3], ps1_pool, copy_engines=mix, chs=(1,))
        phase2([5], ps2_pool)
        phase2([6], ps2_pool)
        phase2([7], ps2_pool)

    nc.compile()
    return nc


def _install_profile_hook():
    """Register the NTFF profile hook that the image's boot skipped
    (antenv.axon_hooks shim is missing in this container)."""
    import sys as _sys
    import types as _types
    try:
        import antenv
        if "antenv.axon_hooks" not in _sys.modules:
            m = _types.ModuleType("antenv.axon_hooks")
            _h = [None]
            m.set_axon_ntff_profile_hook = lambda h: _h.__setitem__(0, h)
            m.get_axon_ntff_profile_hook = lambda: _h[0]
            _sys.modules["antenv.axon_hooks"] = m
            antenv.axon_hooks = m
        from antenv.axon_hooks import set_axon_ntff_profile_hook, get_axon_ntff_profile_hook
        if get_axon_ntff_profile_hook() is None:
            from trn_agent_boot.trn_boot import _ntff_profile_via_ctypes
            set_axon_ntff_profile_hook(_ntff_profile_via_ctypes("/opt/axon/libaxon_pjrt.so"))
    except Exception:
        pass


def _patch_walrus_ldw_opt():
    """Enable walrus LDWEIGHTS dedup (concourse hardcodes it off). With fp32r
    matmuls walrus emits one LDWEIGHTS per matmul; repeated identical loads
    serialize against the matmul stream (same-row-group loads can't pull
    ahead), costing ~170-300ns per matmul."""
    import os
    if os.environ.get("KERNEL_LDW_OPT", "0") != "1":
        return
    import concourse.bass_utils as bu
    if getattr(bu.run_command, "_ldw_patched", False):
        return
    orig = bu.run_command

    def patched(argv, **kw):
        argv = ["--enable-ldw-opt=true" if a == "--enable-ldw-opt=false" else a
                for a in argv]
        return orig(argv, **kw)

    patched._ldw_patched = True
    bu.run_command = patched


def run(inputs, trace=False, repeats=1):
    """Returns (full_output, BassKernelResults)."""
    from concourse.bass_utils import run_bass_kernel_spmd

    _patch_walrus_ldw_opt()
    if trace:
        _install_profile_hook()
    per_core = _host_prep(**inputs)
    nc = _build_program()
    import os as _os
    _tc = [int(x) for x in _os.environ.get("KERNEL_TRACE_CORES", "0").split(",")]
    times = []
    for r in range(repeats):
        res = run_bass_kernel_spmd(nc, per_core, list(range(NCORES)), trace=trace,
                                   trace_cores=_tc if trace else None)
        if res.exec_time_ns is not None:
            times.append(res.exec_time_ns)
    if times:
        res.all_exec_times_ns = times
    # per-core scratch is (B, S, KV, S) with k-major planes: swap to (B,S,S,KV)
    out = np.concatenate(
        [res.results[c]["out"].astype(np.float32).transpose(0, 1, 3, 2)
         for c in range(NCORES)], axis=3)
    out = np.ascontiguousarray(out)
    return out, res


def kernel(**inputs) -> np.ndarray:
    out, _ = run(inputs, trace=False)
    return out
